# revision 2
# baseline (speedup 1.0000x reference)
"""ATMFormer block — data-parallel over batch across 8 NeuronCores.

Sharding: B=8 images, one per core (windows are fully independent across
images); all weights replicated. Each core runs the full block (LN ->
bidirectional window attention + motion head -> MLP with 3x3 depthwise
conv) on its image; outputs are gathered back to full shape on host.
"""

import numpy as np

WS = 7
NUM_HEADS = 8
B, HIMG, WIMG, C = 8, 112, 112, 128
HID = 4 * C

WEIGHT_KEYS = (
    "ln1_g", "ln1_b", "Wq", "Wkv", "Wp", "bp", "Wm1", "bm1", "Wm2", "bm2",
    "ln2_g", "ln2_b", "W1", "b1", "dwk", "dwb", "W2", "b2",
)


def _build_block(jnp, jax):
    def _window_partition(x, ws):
        b, h, w, c = x.shape
        x = x.reshape(b, h // ws, ws, w // ws, ws, c)
        return x.transpose(0, 1, 3, 2, 4, 5).reshape(-1, ws * ws, c)

    def _window_reverse(win, ws, h, w):
        nwb, n, c = win.shape
        b = nwb // ((h // ws) * (w // ws))
        x = win.reshape(b, h // ws, w // ws, ws, ws, c)
        return x.transpose(0, 1, 3, 2, 4, 5).reshape(b, h, w, c)

    def _layernorm(x, g, b):
        mu = x.mean(-1, keepdims=True)
        var = ((x - mu) ** 2).mean(-1, keepdims=True)
        return (x - mu) * jax.lax.rsqrt(var + 1e-5) * g + b

    def _rel_coord(ws):
        idx = np.arange(ws * ws)
        px = (idx % ws).astype(np.float32)
        py = (idx // ws).astype(np.float32)
        return jnp.asarray(
            np.stack([px[None, :] - px[:, None], py[None, :] - py[:, None]], 0)
        )

    def _attn_to_motion(xq, xkv, Wq, Wkv, Wp, bp, Wm1, bm1, Wm2, bm2, rel):
        bw, n, c = xq.shape
        hd = c // NUM_HEADS
        q = (xq @ Wq).reshape(bw, n, NUM_HEADS, hd).transpose(0, 2, 1, 3)
        kv = (xkv @ Wkv).reshape(bw, n, 2, NUM_HEADS, hd).transpose(2, 0, 3, 1, 4)
        k, v = kv[0], kv[1]
        attn = jax.nn.softmax(
            jnp.einsum("bhqd,bhkd->bhqk", q, k) * hd**-0.5, axis=-1
        )
        x = jnp.einsum("bhqk,bhkd->bqhd", attn, v).reshape(bw, n, c)
        x = x @ Wp + bp
        m = jnp.einsum("bhqk,cqk->bqch", attn, rel)
        m = jax.nn.gelu(m @ Wm1 + bm1, approximate=False)
        m = (m @ Wm2 + bm2)[..., 0]
        return x, m

    def _mlp(x, h, w, W1, b1, dwk, dwb, W2, b2):
        b, n, c = x.shape
        hid = x @ W1 + b1
        ch = hid.shape[-1]
        img = hid.transpose(0, 2, 1).reshape(b, ch, h, w)
        img = jax.lax.conv_general_dilated(
            img, dwk, (1, 1), "SAME", feature_group_count=ch,
            dimension_numbers=("NCHW", "OIHW", "NCHW"),
        )
        img = img + dwb[None, :, None, None]
        hid = jax.nn.gelu(img.reshape(b, ch, n).transpose(0, 2, 1), approximate=False)
        return hid @ W2 + b2

    def block(x1, x2, ln1_g, ln1_b, Wq, Wkv, Wp, bp, Wm1, bm1, Wm2, bm2,
              ln2_g, ln2_b, W1, b1, dwk, dwb, W2, b2):
        b, h, w, c = x1.shape
        rel = _rel_coord(WS)
        n1 = _layernorm(x1, ln1_g, ln1_b)
        n2 = _layernorm(x2, ln1_g, ln1_b)
        w1 = _window_partition(n1, WS)
        w2 = _window_partition(n2, WS)
        o1, m1 = _attn_to_motion(w1, w2, Wq, Wkv, Wp, bp, Wm1, bm1, Wm2, bm2, rel)
        o2, m2 = _attn_to_motion(w2, w1, Wq, Wkv, Wp, bp, Wm1, bm1, Wm2, bm2, rel)

        def branch(x, o):
            xo = x + _window_reverse(o, WS, h, w)
            xf = xo.reshape(b, h * w, c)
            xf = xf + _mlp(_layernorm(xf, ln2_g, ln2_b), h, w, W1, b1, dwk, dwb, W2, b2)
            return xf.reshape(b, h, w, c)

        y1 = branch(x1, o1)
        y2 = branch(x2, o2)
        motion = jnp.concatenate(
            [_window_reverse(m1, WS, h, w), _window_reverse(m2, WS, h, w)], -1
        )
        return y1, y2, motion

    return block


def _run_sharded_neuron(inputs):
    """One image per NeuronCore via pmap over the batch axis."""
    import jax
    import jax.numpy as jnp

    devs = [d for d in jax.devices() if d.platform != "cpu"][:8]
    if len(devs) < 8:
        raise RuntimeError(f"need 8 accelerator devices, got {len(devs)}")

    block = _build_block(jnp, jax)

    # per-device batch of 1: axis 0 = device, axis 1 = local batch
    x1 = np.asarray(inputs["x1"], np.float32).reshape(8, 1, HIMG, WIMG, C)
    x2 = np.asarray(inputs["x2"], np.float32).reshape(8, 1, HIMG, WIMG, C)
    weights = {k: np.asarray(inputs[k], np.float32) for k in WEIGHT_KEYS}

    fn = jax.pmap(
        lambda a, b, w: block(a, b, **w),
        axis_name="i",
        in_axes=(0, 0, None),
        devices=devs,
    )
    y1, y2, motion = fn(x1, x2, weights)
    y1 = np.asarray(y1).reshape(B, HIMG, WIMG, C)
    y2 = np.asarray(y2).reshape(B, HIMG, WIMG, C)
    motion = np.asarray(motion).reshape(B, HIMG, WIMG, 4)
    return y1, y2, motion


def _run_cpu(inputs):
    import jax
    import jax.numpy as jnp

    cpu = jax.devices("cpu")[0]
    block = _build_block(jnp, jax)
    with jax.default_device(cpu):
        args = {k: jnp.asarray(np.asarray(v), jnp.float32) for k, v in inputs.items()}
        y1, y2, motion = jax.jit(block)(**args)
    return np.asarray(y1), np.asarray(y2), np.asarray(motion)


def kernel(**inputs):
    import signal

    alarm_armed = False
    old_handler = None
    try:
        # Guard against a hung device compile: fall back to CPU rather than
        # blocking forever. Only usable from the main thread.
        def _raise_timeout(signum, frame):
            raise TimeoutError("neuron path exceeded time budget")

        try:
            old_handler = signal.signal(signal.SIGALRM, _raise_timeout)
            signal.alarm(1200)
            alarm_armed = True
        except (ValueError, OSError):
            pass  # non-main thread: run unguarded

        out = _run_sharded_neuron(inputs)
        return out
    except Exception as e:  # pragma: no cover - device-environment dependent
        import sys
        print(f"kernel: neuron path failed ({type(e).__name__}: {e}); "
              "falling back to CPU", file=sys.stderr)
        return _run_cpu(inputs)
    finally:
        if alarm_armed:
            signal.alarm(0)
            signal.signal(signal.SIGALRM, old_handler)
